# revision 33
# baseline (speedup 1.0000x reference)
"""Trainium2 Bass kernel for nn_CustomConv2d: 3x3 conv, stride 1, pad 1.

Full shapes: x (32,128,56,56) f32, weight (256,128,3,3) f32, bias (256,) f32.
Output: (32,256,56,56) f32.

Strategy: data-parallel over batch (8 cores x 4 images) + 1D Winograd F(2,3)
along W in bf16. Per output column-pair only 4 products are needed instead of
6, cutting PE matmul work to 2/3 of the direct conv: per (image, cout-half,
14-row chunk) the kernel runs 12 accumulating bf16 matmuls (4 Winograd
components x 3 ky taps, contraction = Cin = 128, free = 14 rows x 28 pairs =
392) instead of 9 direct taps over all 56 columns. Input transform (E/T/U
full-width row combos) runs on DVE at the 2x bf16-packed rate. Reconstruction
(out0 = m1+m2+m3+bias at even cols, out1 = m2-m3-m4+bias at odd) respects the
engine PSUM rules (DVE max one PSUM input, Pool none): ACT drains s2=m2+bias,
s3=m3, s4=m4 to SBUF bf16, DVE computes u=s2+s3 and the single trailing op
och0=u+m1, Pool computes v=s2-s3 and och1=v-s4. Group order m2,m3,m4,m1 lets
recon overlap the matmul stream. x is cast to bf16 on the host (halves input
DMA); outputs store as bf16 and upcast on the host (halves output DMA); G
combos live in a [cin][t][12][128] layout so every weight DMA is contiguous
(strided DMAs cost ~2x per byte). Max rel err 7.0e-3 vs the f32 reference
(gate 2e-2). Dep-free warmup matmuls bridge the initial DMA wait and the PE
clock ramp. Cost-model timeline: 72299 ns (baseline direct-conv f32r: 104438).
"""

import numpy as np
import ml_dtypes

import concourse.bass as bass
import concourse.mybir as mybir
import concourse.tile as tile
from concourse import bacc
from concourse.bass_utils import run_bass_kernel_spmd

N_CORES = 8
B = 32
B_LOC = B // N_CORES  # 4
CIN = 128
COUT = 256
H = W = 56
HP = WP = 58  # padded
RCH = 14  # output rows per chunk
NCH = H // RCH  # 4
J = W // 2  # 28 column pairs
NWARM = 9

_NC_CACHE = None
LAST_RESULTS = None  # stashed BassKernelResults for test harness introspection


def _build(reps: int = 1) -> bass.Bass:
    f32 = mybir.dt.float32
    bf16 = mybir.dt.bfloat16
    alu = mybir.AluOpType
    nc = bacc.Bacc(None, target_bir_lowering=False)
    x_d = nc.dram_tensor("x", [B_LOC, CIN, HP * WP], bf16, kind="ExternalInput")
    g_d = nc.dram_tensor("g", [CIN, 2 * 12 * 128], bf16, kind="ExternalInput")
    b_d = nc.dram_tensor("b", [2, 128], f32, kind="ExternalInput")
    y_d = nc.dram_tensor("y", [B_LOC, COUT, H * W], bf16, kind="ExternalOutput")

    g3 = g_d[:].rearrange("p (t i o) -> p t i o", t=2, i=12)

    from contextlib import ExitStack, nullcontext

    with tile.TileContext(nc) as tc, ExitStack() as es:
        cpool = es.enter_context(tc.tile_pool(name="const", bufs=1))
        xpool = es.enter_context(tc.tile_pool(name="xp", bufs=B_LOC))
        vpool = es.enter_context(tc.tile_pool(name="vp", bufs=3))
        tpool = es.enter_context(tc.tile_pool(name="tmp", bufs=4))
        opool = es.enter_context(tc.tile_pool(name="out", bufs=6))
        pspool = es.enter_context(tc.tile_pool(name="ps", bufs=8, space="PSUM"))
        with tc.For_i(0, reps, 1) if reps > 1 else nullcontext():
            wtile = cpool.tile([CIN, 2, 12, 128], bf16)
            btile = cpool.tile([128, 2], f32)
            xpads = [
                xpool.tile([CIN, HP, WP], bf16, tag="xpad", name=f"xpad{i}")
                for i in range(B_LOC)
            ]

            # PE warmup: dep-free matmuls bridge the initial DMA wait and
            # bring the PE clock (HAM) to full rate before the real work.
            wsrc = cpool.tile([128, RCH * J], bf16)
            nc.gpsimd.memset(wsrc[:], 0.0)
            wps = pspool.tile([128, RCH * J], f32, tag="m")
            for _ in range(NWARM):
                nc.tensor.matmul(
                    wps[0:64, :], wsrc[:, 0:64], wsrc[:], start=True, stop=True
                )

            # DMA issue order = criticality: chunk0's V transform needs xpad
            # rows 0..15 first; the first matmul group needs G[:, 0:3, 0:128].
            xsrc0 = x_d[0].rearrange("p (h w) -> p h w", h=HP)
            nc.sync.dma_start(xpads[0][:, 0:16, :], xsrc0[:, 0:16, :])
            nc.sync.dma_start(wtile[:, 0, 3:12, :], g3[:, 0, 3:12, :])
            nc.sync.dma_start(wtile[:, 0, 0:3, :], g3[:, 0, 0:3, :])
            nc.sync.dma_start(wtile[:, 1], g3[:, 1])
            nc.sync.dma_start(btile[:], b_d[:].rearrange("t p -> p t"))
            nc.sync.dma_start(xpads[0][:, 16:37, :], xsrc0[:, 16:37, :])
            nc.sync.dma_start(xpads[0][:, 37:58, :], xsrc0[:, 37:58, :])
            for b in range(1, B_LOC):
                xsrc = x_d[b].rearrange("p (h w) -> p h w", h=HP)
                nc.sync.dma_start(xpads[b][:, 0:29, :], xsrc[:, 0:29, :])
                nc.sync.dma_start(xpads[b][:, 29:58, :], xsrc[:, 29:58, :])

            def make_v(b, k):
                """E/T/U row transforms for chunk k of image b (DVE, bf16 2x)."""
                xp = xpads[b]
                r0 = k * RCH
                vt = vpool.tile([CIN, 3, RCH + 2, W], bf16, tag="v")
                rs = slice(r0, r0 + RCH + 2)
                # issue order matches first use: T (m2 group runs first),
                # then U (m3), then E (m4/m1)
                nc.vector.tensor_tensor(
                    vt[:, 1], xp[:, rs, 1:57], xp[:, rs, 2:58], alu.add
                )
                nc.vector.tensor_tensor(
                    vt[:, 2], xp[:, rs, 2:58], xp[:, rs, 1:57], alu.subtract
                )
                nc.vector.tensor_tensor(
                    vt[:, 0], xp[:, rs, 0:56], xp[:, rs, 2:58], alu.subtract
                )
                return vt

            # (m-index, v-component, column phase); group order m2,m3,m4,m1:
            # ACT drains s2 = m2+bias and s3 = m3 to SBUF mid-stream (the HW
            # allows only ONE PSUM input per vector op), DVE/Pool combine
            # u = s2+s3, v = s2-s3, och1 = v-m4; only och0 = u+m1 trails.
            SRCS = [(1, 1, 0), (2, 2, 0), (3, 0, 1), (0, 0, 0)]

            def chunk_t(b, k, t, vv, sr, nr, mode):
                """One matmul+recon unit over rows sr..sr+nr of chunk k.
                mode 2 = final unit (recon tuned for trailing latency)."""
                r0 = k * RCH
                ms = [None] * 4
                for i, vc, ph in SRCS:
                    ms[i] = pspool.tile(
                        [CIN, nr, J], f32, tag="m", name=f"m{b}_{k}_{t}_{sr}_{i}"
                    )
                    for ky in range(3):
                        nc.tensor.matmul(
                            ms[i][:],
                            wtile[:, t, i * 3 + ky, :],
                            vv[:, vc, sr + ky : sr + ky + nr, :, ph],
                            start=(ky == 0),
                            stop=(ky == 2),
                        )
                yield  # caller interleaves V prefetch here
                och = opool.tile([128, nr, J, 2], bf16, tag="och")
                s2 = tpool.tile([128, nr, J], bf16, tag="s2")
                s3 = tpool.tile([128, nr, J], bf16, tag="s3")
                u = tpool.tile([128, nr, J], bf16, tag="u")
                v = tpool.tile([128, nr, J], bf16, tag="vv")
                bias = btile[:, t : t + 1]
                nc.scalar.activation(
                    s2[:], ms[1][:], mybir.ActivationFunctionType.Identity, bias=bias
                )
                nc.scalar.activation(
                    s3[:], ms[2][:], mybir.ActivationFunctionType.Identity
                )
                nc.vector.tensor_tensor(u[:], s2[:], s3[:], alu.add)
                s4 = tpool.tile([128, nr, J], bf16, tag="s4")
                if mode == 2:
                    # final unit: v on (idle) Pool, och0 then och1 on DVE so
                    # the trailing chain after the last matmul is minimal
                    nc.gpsimd.tensor_tensor(v[:], s2[:], s3[:], alu.subtract)
                    nc.scalar.activation(
                        s4[:], ms[3][:], mybir.ActivationFunctionType.Identity
                    )
                    nc.vector.tensor_tensor(och[:, :, :, 0], u[:], ms[0][:], alu.add)
                    nc.vector.tensor_tensor(
                        och[:, :, :, 1], v[:], s4[:], alu.subtract
                    )
                else:
                    nc.gpsimd.tensor_tensor(v[:], s2[:], s3[:], alu.subtract)
                    nc.scalar.activation(
                        s4[:], ms[3][:], mybir.ActivationFunctionType.Identity
                    )
                    nc.gpsimd.tensor_tensor(
                        och[:, :, :, 1], v[:], s4[:], alu.subtract
                    )
                    nc.vector.tensor_tensor(och[:, :, :, 0], u[:], ms[0][:], alu.add)
                nc.sync.dma_start(
                    y_d[
                        b,
                        t * 128 : (t + 1) * 128,
                        (r0 + sr) * W : (r0 + sr + nr) * W,
                    ],
                    och[:].rearrange("p r j two -> p (r j two)"),
                )
                yield

            vt = make_v(0, 0)
            for b in range(B_LOC):
                for k in range(NCH):
                    vv = vt[:].rearrange("p v r (j two) -> p v r j two", two=2)
                    final = (b, k) == (B_LOC - 1, NCH - 1)
                    for t in range(2):
                        mode = (2 if t == 1 else 1) if final else 0
                        it = chunk_t(b, k, t, vv, 0, RCH, mode)
                        next(it)
                        if t == 0:
                            # prefetch next chunk's V while this chunk drains
                            nb, nk = (b, k + 1) if k + 1 < NCH else (b + 1, 0)
                            if nb < B_LOC:
                                nvt = make_v(nb, nk)
                        for u in it:
                            pass
                    vt = nvt if not final else None
    nc.finalize()
    return nc


def _bf16(a: np.ndarray) -> np.ndarray:
    return a.astype(ml_dtypes.bfloat16)


def kernel(x, weight, bias, approximate):
    """Full (unsharded) conv2d. `approximate` only selects the HW approximation
    level in the original module; the exact-math output is independent of it."""
    global _NC_CACHE, LAST_RESULTS
    x = np.ascontiguousarray(x, dtype=np.float32)
    weight = np.ascontiguousarray(weight, dtype=np.float32)
    bias = np.ascontiguousarray(bias, dtype=np.float32)

    # host: cast to bf16, zero-pad spatially; shard batch across cores
    xp = np.zeros((B, CIN, HP, WP), ml_dtypes.bfloat16)
    xp[:, :, 1 : H + 1, 1 : W + 1] = _bf16(x)
    xp = xp.reshape(B, CIN, HP * WP)

    # Winograd F(2,3) weight combos (f64 accumulate, single bf16 rounding)
    w64 = weight.astype(np.float64)  # (Cout, Cin, ky, kx)
    w0, w1, w2 = w64[:, :, :, 0], w64[:, :, :, 1], w64[:, :, :, 2]
    combos = [w0, (w0 + w1 + w2) / 2, (w0 - w1 + w2) / 2, w2]
    g = np.empty((CIN, 2, 4, 3, 128), np.float64)
    for i, cb in enumerate(combos):  # cb: (Cout, Cin, ky)
        gt = cb.transpose(1, 2, 0)  # (Cin, ky, Cout)
        g[:, 0, i, :, :] = gt[:, :, 0:128]
        g[:, 1, i, :, :] = gt[:, :, 128:256]
    g2 = np.ascontiguousarray(_bf16(g.reshape(CIN, 2 * 12 * 128)))
    b2 = bias.reshape(2, 128)

    if _NC_CACHE is None:
        _NC_CACHE = _build()
    nc = _NC_CACHE

    in_maps = [
        {"x": xp[c * B_LOC : (c + 1) * B_LOC], "g": g2, "b": b2}
        for c in range(N_CORES)
    ]
    try:
        res = run_bass_kernel_spmd(nc, in_maps, core_ids=list(range(N_CORES)))
    except Exception:
        # transient device-acquisition races (NRT_EXEC_UNIT_UNRECOVERABLE on
        # first touch after a prior process teardown) recover on retry
        import time as _time

        _time.sleep(5.0)
        res = run_bass_kernel_spmd(nc, in_maps, core_ids=list(range(N_CORES)))
    LAST_RESULTS = res
    out = np.concatenate([np.asarray(r["y"]) for r in res.results], axis=0)
    return out.astype(np.float32).reshape(B, COUT, H, W)


# revision 36
# speedup vs baseline: 1.0020x; 1.0020x over previous
"""Trainium2 Bass kernel for nn_CustomConv2d: 3x3 conv, stride 1, pad 1.

Full shapes: x (32,128,56,56) f32, weight (256,128,3,3) f32, bias (256,) f32.
Output: (32,256,56,56) f32.

Strategy: data-parallel over batch (8 cores x 4 images) + 1D Winograd F(2,3)
along W in bf16. Per output column-pair only 4 products are needed instead of
6, cutting PE matmul work to 2/3 of the direct conv: per (image, cout-half,
14-row chunk) the kernel runs 12 accumulating bf16 matmuls (4 Winograd
components x 3 ky taps, contraction = Cin = 128, free = 14 rows x 28 pairs =
392) instead of 9 direct taps over all 56 columns. Input transform (E/T/U
full-width row combos) runs on DVE at the 2x bf16-packed rate. Reconstruction
(out0 = m1+m2+m3+bias at even cols, out1 = m2-m3-m4+bias at odd) respects the
engine PSUM rules (DVE max one PSUM input, Pool none): ACT drains s2=m2+bias,
s3=m3, s4=m4 to SBUF bf16, DVE computes u=s2+s3 and the single trailing op
och0=u+m1, Pool computes v=s2-s3 and och1=v-s4. Group order m2,m3,m4,m1 lets
recon overlap the matmul stream. x is cast to bf16 on the host (halves input
DMA); outputs store as bf16 and upcast on the host (halves output DMA); G
combos live in a [cin][t][12][128] layout so every weight DMA is contiguous
(strided DMAs cost ~2x per byte). Max rel err 7.0e-3 vs the f32 reference
(gate 2e-2). Dep-free warmup matmuls bridge the initial DMA wait and the PE
clock ramp. Cost-model timeline: 72299 ns (baseline direct-conv f32r: 104438).
"""

import numpy as np
import ml_dtypes

import concourse.bass as bass
import concourse.mybir as mybir
import concourse.tile as tile
from concourse import bacc
from concourse.bass_utils import run_bass_kernel_spmd

N_CORES = 8
B = 32
B_LOC = B // N_CORES  # 4
CIN = 128
COUT = 256
H = W = 56
HP = WP = 58  # padded
RCH = 14  # output rows per chunk
NCH = H // RCH  # 4
J = W // 2  # 28 column pairs
NWARM = 8

_NC_CACHE = None
LAST_RESULTS = None  # stashed BassKernelResults for test harness introspection


def _build(reps: int = 1) -> bass.Bass:
    f32 = mybir.dt.float32
    bf16 = mybir.dt.bfloat16
    alu = mybir.AluOpType
    nc = bacc.Bacc(None, target_bir_lowering=False)
    x_d = nc.dram_tensor("x", [B_LOC, CIN, HP * WP], bf16, kind="ExternalInput")
    g_d = nc.dram_tensor("g", [CIN, 2 * 12 * 128], bf16, kind="ExternalInput")
    b_d = nc.dram_tensor("b", [2, 128], f32, kind="ExternalInput")
    y_d = nc.dram_tensor("y", [B_LOC, COUT, H * W], bf16, kind="ExternalOutput")

    g3 = g_d[:].rearrange("p (t i o) -> p t i o", t=2, i=12)

    from contextlib import ExitStack, nullcontext

    with tile.TileContext(nc) as tc, ExitStack() as es:
        cpool = es.enter_context(tc.tile_pool(name="const", bufs=1))
        xpool = es.enter_context(tc.tile_pool(name="xp", bufs=B_LOC))
        vpool = es.enter_context(tc.tile_pool(name="vp", bufs=3))
        tpool = es.enter_context(tc.tile_pool(name="tmp", bufs=4))
        opool = es.enter_context(tc.tile_pool(name="out", bufs=6))
        pspool = es.enter_context(tc.tile_pool(name="ps", bufs=8, space="PSUM"))
        with tc.For_i(0, reps, 1) if reps > 1 else nullcontext():
            wtile = cpool.tile([CIN, 2, 12, 128], bf16)
            btile = cpool.tile([128, 2], f32)
            xpads = [
                xpool.tile([CIN, HP, WP], bf16, tag="xpad", name=f"xpad{i}")
                for i in range(B_LOC)
            ]

            # PE warmup: dep-free matmuls bridge the initial DMA wait and
            # bring the PE clock (HAM) to full rate before the real work.
            wsrc = cpool.tile([128, RCH * J], bf16)
            nc.gpsimd.memset(wsrc[:], 0.0)
            wps = pspool.tile([128, RCH * J], f32, tag="m")
            for _ in range(NWARM):
                nc.tensor.matmul(
                    wps[0:64, :], wsrc[:, 0:64], wsrc[:], start=True, stop=True
                )

            # DMA issue order = criticality: chunk0's V transform needs xpad
            # rows 0..15 first; the first matmul group needs G[:, 0:3, 0:128].
            xsrc0 = x_d[0].rearrange("p (h w) -> p h w", h=HP)
            nc.sync.dma_start(xpads[0][:, 0:16, :], xsrc0[:, 0:16, :])
            nc.sync.dma_start(wtile[:, 0, 3:12, :], g3[:, 0, 3:12, :])
            nc.sync.dma_start(wtile[:, 0, 0:3, :], g3[:, 0, 0:3, :])
            nc.sync.dma_start(wtile[:, 1], g3[:, 1])
            nc.sync.dma_start(btile[:], b_d[:].rearrange("t p -> p t"))
            nc.sync.dma_start(xpads[0][:, 16:37, :], xsrc0[:, 16:37, :])
            nc.sync.dma_start(xpads[0][:, 37:58, :], xsrc0[:, 37:58, :])
            for b in range(1, B_LOC):
                xsrc = x_d[b].rearrange("p (h w) -> p h w", h=HP)
                nc.sync.dma_start(xpads[b][:, 0:29, :], xsrc[:, 0:29, :])
                nc.sync.dma_start(xpads[b][:, 29:58, :], xsrc[:, 29:58, :])

            def make_v(b, k):
                """E/T/U row transforms for chunk k of image b (DVE, bf16 2x)."""
                xp = xpads[b]
                r0 = k * RCH
                vt = vpool.tile([CIN, 3, RCH + 2, W], bf16, tag="v")
                rs = slice(r0, r0 + RCH + 2)
                # issue order matches first use: T (m2 group runs first),
                # then U (m3), then E (m4/m1)
                nc.vector.tensor_tensor(
                    vt[:, 1], xp[:, rs, 1:57], xp[:, rs, 2:58], alu.add
                )
                nc.vector.tensor_tensor(
                    vt[:, 2], xp[:, rs, 2:58], xp[:, rs, 1:57], alu.subtract
                )
                nc.vector.tensor_tensor(
                    vt[:, 0], xp[:, rs, 0:56], xp[:, rs, 2:58], alu.subtract
                )
                return vt

            # (m-index, v-component, column phase); group order m2,m3,m4,m1:
            # ACT drains s2 = m2+bias and s3 = m3 to SBUF mid-stream (the HW
            # allows only ONE PSUM input per vector op), DVE/Pool combine
            # u = s2+s3, v = s2-s3, och1 = v-m4; only och0 = u+m1 trails.
            SRCS = [(1, 1, 0), (2, 2, 0), (3, 0, 1), (0, 0, 0)]

            def chunk_t(b, k, t, vv, sr, nr, mode):
                """One matmul+recon unit over rows sr..sr+nr of chunk k.
                mode 2 = final unit (recon tuned for trailing latency)."""
                r0 = k * RCH
                ms = [None] * 4
                for i, vc, ph in SRCS:
                    ms[i] = pspool.tile(
                        [CIN, nr, J], f32, tag="m", name=f"m{b}_{k}_{t}_{sr}_{i}"
                    )
                    for ky in range(3):
                        nc.tensor.matmul(
                            ms[i][:],
                            wtile[:, t, i * 3 + ky, :],
                            vv[:, vc, sr + ky : sr + ky + nr, :, ph],
                            start=(ky == 0),
                            stop=(ky == 2),
                        )
                yield  # caller interleaves V prefetch here
                och = opool.tile([128, nr, J, 2], bf16, tag="och")
                s2 = tpool.tile([128, nr, J], bf16, tag="s2")
                s3 = tpool.tile([128, nr, J], bf16, tag="s3")
                u = tpool.tile([128, nr, J], bf16, tag="u")
                v = tpool.tile([128, nr, J], bf16, tag="vv")
                bias = btile[:, t : t + 1]
                nc.scalar.activation(
                    s2[:], ms[1][:], mybir.ActivationFunctionType.Identity, bias=bias
                )
                nc.scalar.activation(
                    s3[:], ms[2][:], mybir.ActivationFunctionType.Identity
                )
                nc.vector.tensor_tensor(u[:], s2[:], s3[:], alu.add)
                s4 = tpool.tile([128, nr, J], bf16, tag="s4")
                if mode == 2:
                    # final unit: v early on Pool (mode 1 freed it), och0 on
                    # DVE, och1 split Pool || DVE after the last matmul
                    hr = nr // 2
                    nc.gpsimd.tensor_tensor(v[:], s2[:], s3[:], alu.subtract)
                    nc.scalar.activation(
                        s4[:], ms[3][:], mybir.ActivationFunctionType.Identity
                    )
                    nc.vector.tensor_tensor(och[:, :, :, 0], u[:], ms[0][:], alu.add)
                    nc.gpsimd.tensor_tensor(
                        och[:, 0:hr, :, 1], v[:, 0:hr], s4[:, 0:hr], alu.subtract
                    )
                    nc.vector.tensor_tensor(
                        och[:, hr:nr, :, 1], v[:, hr:nr], s4[:, hr:nr], alu.subtract
                    )
                else:
                    nc.gpsimd.tensor_tensor(v[:], s2[:], s3[:], alu.subtract)
                    nc.scalar.activation(
                        s4[:], ms[3][:], mybir.ActivationFunctionType.Identity
                    )
                    if mode == 1:
                        # near-tail: keep Pool clear for the final unit's v
                        nc.vector.tensor_tensor(
                            och[:, :, :, 1], v[:], s4[:], alu.subtract
                        )
                    else:
                        nc.gpsimd.tensor_tensor(
                            och[:, :, :, 1], v[:], s4[:], alu.subtract
                        )
                    nc.vector.tensor_tensor(och[:, :, :, 0], u[:], ms[0][:], alu.add)
                nc.sync.dma_start(
                    y_d[
                        b,
                        t * 128 : (t + 1) * 128,
                        (r0 + sr) * W : (r0 + sr + nr) * W,
                    ],
                    och[:].rearrange("p r j two -> p (r j two)"),
                )
                yield

            vt = make_v(0, 0)
            for b in range(B_LOC):
                for k in range(NCH):
                    vv = vt[:].rearrange("p v r (j two) -> p v r j two", two=2)
                    final = (b, k) == (B_LOC - 1, NCH - 1)
                    for t in range(2):
                        mode = (2 if t == 1 else 1) if final else 0
                        it = chunk_t(b, k, t, vv, 0, RCH, mode)
                        next(it)
                        if t == 0:
                            # prefetch next chunk's V while this chunk drains
                            nb, nk = (b, k + 1) if k + 1 < NCH else (b + 1, 0)
                            if nb < B_LOC:
                                nvt = make_v(nb, nk)
                        for u in it:
                            pass
                    vt = nvt if not final else None
    nc.finalize()
    return nc


def _bf16(a: np.ndarray) -> np.ndarray:
    return a.astype(ml_dtypes.bfloat16)


def kernel(x, weight, bias, approximate):
    """Full (unsharded) conv2d. `approximate` only selects the HW approximation
    level in the original module; the exact-math output is independent of it."""
    global _NC_CACHE, LAST_RESULTS
    x = np.ascontiguousarray(x, dtype=np.float32)
    weight = np.ascontiguousarray(weight, dtype=np.float32)
    bias = np.ascontiguousarray(bias, dtype=np.float32)

    # host: cast to bf16, zero-pad spatially; shard batch across cores
    xp = np.zeros((B, CIN, HP, WP), ml_dtypes.bfloat16)
    xp[:, :, 1 : H + 1, 1 : W + 1] = _bf16(x)
    xp = xp.reshape(B, CIN, HP * WP)

    # Winograd F(2,3) weight combos (f64 accumulate, single bf16 rounding)
    w64 = weight.astype(np.float64)  # (Cout, Cin, ky, kx)
    w0, w1, w2 = w64[:, :, :, 0], w64[:, :, :, 1], w64[:, :, :, 2]
    combos = [w0, (w0 + w1 + w2) / 2, (w0 - w1 + w2) / 2, w2]
    g = np.empty((CIN, 2, 4, 3, 128), np.float64)
    for i, cb in enumerate(combos):  # cb: (Cout, Cin, ky)
        gt = cb.transpose(1, 2, 0)  # (Cin, ky, Cout)
        g[:, 0, i, :, :] = gt[:, :, 0:128]
        g[:, 1, i, :, :] = gt[:, :, 128:256]
    g2 = np.ascontiguousarray(_bf16(g.reshape(CIN, 2 * 12 * 128)))
    b2 = bias.reshape(2, 128)

    if _NC_CACHE is None:
        _NC_CACHE = _build()
    nc = _NC_CACHE

    in_maps = [
        {"x": xp[c * B_LOC : (c + 1) * B_LOC], "g": g2, "b": b2}
        for c in range(N_CORES)
    ]
    try:
        res = run_bass_kernel_spmd(nc, in_maps, core_ids=list(range(N_CORES)))
    except Exception:
        # transient device-acquisition races (NRT_EXEC_UNIT_UNRECOVERABLE on
        # first touch after a prior process teardown) recover on retry
        import time as _time

        _time.sleep(5.0)
        res = run_bass_kernel_spmd(nc, in_maps, core_ids=list(range(N_CORES)))
    LAST_RESULTS = res
    out = np.concatenate([np.asarray(r["y"]) for r in res.results], axis=0)
    return out.astype(np.float32).reshape(B, COUT, H, W)


# revision 38
# speedup vs baseline: 1.0055x; 1.0034x over previous
"""Trainium2 Bass kernel for nn_CustomConv2d: 3x3 conv, stride 1, pad 1.

Full shapes: x (32,128,56,56) f32, weight (256,128,3,3) f32, bias (256,) f32.
Output: (32,256,56,56) f32.

Strategy: data-parallel over batch (8 cores x 4 images) + 1D Winograd F(2,3)
along W in bf16. Per output column-pair only 4 products are needed instead of
6, cutting PE matmul work to 2/3 of the direct conv: per (image, cout-half,
14-row chunk) the kernel runs 12 accumulating bf16 matmuls (4 Winograd
components x 3 ky taps, contraction = Cin = 128, free = 14 rows x 28 pairs =
392) instead of 9 direct taps over all 56 columns. Input transform (E/T/U
full-width row combos) runs on DVE at the 2x bf16-packed rate. Reconstruction
(out0 = m1+m2+m3+bias at even cols, out1 = m2-m3-m4+bias at odd) respects the
engine PSUM rules (DVE max one PSUM input, Pool none): ACT drains s2=m2+bias,
s3=m3, s4=m4 to SBUF bf16, DVE computes u=s2+s3 and the single trailing op
och0=u+m1, Pool computes v=s2-s3 and och1=v-s4. Group order m2,m3,m4,m1 lets
recon overlap the matmul stream. x is cast to bf16 on the host (halves input
DMA); outputs store as bf16 and upcast on the host (halves output DMA); G
combos live in a [cin][t][12][128] layout so every weight DMA is contiguous
(strided DMAs cost ~2x per byte). Max rel err 7.0e-3 vs the f32 reference
(gate 2e-2). Dep-free warmup matmuls bridge the initial DMA wait and the PE
clock ramp; the final chunk-t runs as two half-units so the trailing recon
and last store are half-size. Cost-model timeline: 71906 ns (baseline
direct-conv f32r: 104438).
"""

import numpy as np
import ml_dtypes

import concourse.bass as bass
import concourse.mybir as mybir
import concourse.tile as tile
from concourse import bacc
from concourse.bass_utils import run_bass_kernel_spmd

N_CORES = 8
B = 32
B_LOC = B // N_CORES  # 4
CIN = 128
COUT = 256
H = W = 56
HP = WP = 58  # padded
RCH = 14  # output rows per chunk
NCH = H // RCH  # 4
J = W // 2  # 28 column pairs
NWARM = 8

_NC_CACHE = None
LAST_RESULTS = None  # stashed BassKernelResults for test harness introspection


def _build(reps: int = 1) -> bass.Bass:
    f32 = mybir.dt.float32
    bf16 = mybir.dt.bfloat16
    alu = mybir.AluOpType
    nc = bacc.Bacc(None, target_bir_lowering=False)
    x_d = nc.dram_tensor("x", [B_LOC, CIN, HP * WP], bf16, kind="ExternalInput")
    g_d = nc.dram_tensor("g", [CIN, 2 * 12 * 128], bf16, kind="ExternalInput")
    b_d = nc.dram_tensor("b", [2, 128], f32, kind="ExternalInput")
    y_d = nc.dram_tensor("y", [B_LOC, COUT, H * W], bf16, kind="ExternalOutput")

    g3 = g_d[:].rearrange("p (t i o) -> p t i o", t=2, i=12)

    from contextlib import ExitStack, nullcontext

    with tile.TileContext(nc) as tc, ExitStack() as es:
        cpool = es.enter_context(tc.tile_pool(name="const", bufs=1))
        xpool = es.enter_context(tc.tile_pool(name="xp", bufs=B_LOC))
        vpool = es.enter_context(tc.tile_pool(name="vp", bufs=3))
        tpool = es.enter_context(tc.tile_pool(name="tmp", bufs=4))
        opool = es.enter_context(tc.tile_pool(name="out", bufs=6))
        pspool = es.enter_context(tc.tile_pool(name="ps", bufs=8, space="PSUM"))
        with tc.For_i(0, reps, 1) if reps > 1 else nullcontext():
            wtile = cpool.tile([CIN, 2, 12, 128], bf16)
            btile = cpool.tile([128, 2], f32)
            xpads = [
                xpool.tile([CIN, HP, WP], bf16, tag="xpad", name=f"xpad{i}")
                for i in range(B_LOC)
            ]

            # PE warmup: dep-free matmuls bridge the initial DMA wait and
            # bring the PE clock (HAM) to full rate before the real work.
            wsrc = cpool.tile([128, RCH * J], bf16)
            nc.gpsimd.memset(wsrc[:], 0.0)
            wps = pspool.tile([128, RCH * J], f32, tag="m")
            for _ in range(NWARM):
                nc.tensor.matmul(
                    wps[0:64, :], wsrc[:, 0:64], wsrc[:], start=True, stop=True
                )

            # DMA issue order = criticality: chunk0's V transform needs xpad
            # rows 0..15 first; the first matmul group needs G[:, 0:3, 0:128].
            xsrc0 = x_d[0].rearrange("p (h w) -> p h w", h=HP)
            nc.sync.dma_start(xpads[0][:, 0:16, :], xsrc0[:, 0:16, :])
            nc.sync.dma_start(wtile[:, 0, 3:12, :], g3[:, 0, 3:12, :])
            nc.sync.dma_start(wtile[:, 0, 0:3, :], g3[:, 0, 0:3, :])
            nc.sync.dma_start(wtile[:, 1], g3[:, 1])
            nc.sync.dma_start(btile[:], b_d[:].rearrange("t p -> p t"))
            nc.sync.dma_start(xpads[0][:, 16:37, :], xsrc0[:, 16:37, :])
            nc.sync.dma_start(xpads[0][:, 37:58, :], xsrc0[:, 37:58, :])
            for b in range(1, B_LOC):
                xsrc = x_d[b].rearrange("p (h w) -> p h w", h=HP)
                nc.sync.dma_start(xpads[b][:, 0:29, :], xsrc[:, 0:29, :])
                nc.sync.dma_start(xpads[b][:, 29:58, :], xsrc[:, 29:58, :])

            def make_v(b, k):
                """E/T/U row transforms for chunk k of image b (DVE, bf16 2x)."""
                xp = xpads[b]
                r0 = k * RCH
                vt = vpool.tile([CIN, 3, RCH + 2, W], bf16, tag="v")
                rs = slice(r0, r0 + RCH + 2)
                # issue order matches first use: T (m2 group runs first),
                # then U (m3), then E (m4/m1)
                nc.vector.tensor_tensor(
                    vt[:, 1], xp[:, rs, 1:57], xp[:, rs, 2:58], alu.add
                )
                nc.vector.tensor_tensor(
                    vt[:, 2], xp[:, rs, 2:58], xp[:, rs, 1:57], alu.subtract
                )
                nc.vector.tensor_tensor(
                    vt[:, 0], xp[:, rs, 0:56], xp[:, rs, 2:58], alu.subtract
                )
                return vt

            # (m-index, v-component, column phase); group order m2,m3,m4,m1:
            # ACT drains s2 = m2+bias and s3 = m3 to SBUF mid-stream (the HW
            # allows only ONE PSUM input per vector op), DVE/Pool combine
            # u = s2+s3, v = s2-s3, och1 = v-m4; only och0 = u+m1 trails.
            SRCS = [(1, 1, 0), (2, 2, 0), (3, 0, 1), (0, 0, 0)]

            def chunk_t(b, k, t, vv, sr, nr, mode):
                """One matmul+recon unit over rows sr..sr+nr of chunk k.
                mode 2 = final unit (recon tuned for trailing latency)."""
                r0 = k * RCH
                ms = [None] * 4
                for i, vc, ph in SRCS:
                    ms[i] = pspool.tile(
                        [CIN, nr, J], f32, tag="m", name=f"m{b}_{k}_{t}_{sr}_{i}"
                    )
                    for ky in range(3):
                        nc.tensor.matmul(
                            ms[i][:],
                            wtile[:, t, i * 3 + ky, :],
                            vv[:, vc, sr + ky : sr + ky + nr, :, ph],
                            start=(ky == 0),
                            stop=(ky == 2),
                        )
                yield  # caller interleaves V prefetch here
                och = opool.tile([128, nr, J, 2], bf16, tag="och")
                s2 = tpool.tile([128, nr, J], bf16, tag="s2")
                s3 = tpool.tile([128, nr, J], bf16, tag="s3")
                u = tpool.tile([128, nr, J], bf16, tag="u")
                v = tpool.tile([128, nr, J], bf16, tag="vv")
                bias = btile[:, t : t + 1]
                nc.scalar.activation(
                    s2[:], ms[1][:], mybir.ActivationFunctionType.Identity, bias=bias
                )
                nc.scalar.activation(
                    s3[:], ms[2][:], mybir.ActivationFunctionType.Identity
                )
                nc.vector.tensor_tensor(u[:], s2[:], s3[:], alu.add)
                s4 = tpool.tile([128, nr, J], bf16, tag="s4")
                if mode == 2:
                    # final unit: v early on Pool (mode 1 freed it), och0 on
                    # DVE, och1 split Pool || DVE after the last matmul
                    hr = nr // 2
                    nc.gpsimd.tensor_tensor(v[:], s2[:], s3[:], alu.subtract)
                    nc.scalar.activation(
                        s4[:], ms[3][:], mybir.ActivationFunctionType.Identity
                    )
                    nc.vector.tensor_tensor(och[:, :, :, 0], u[:], ms[0][:], alu.add)
                    nc.gpsimd.tensor_tensor(
                        och[:, 0:hr, :, 1], v[:, 0:hr], s4[:, 0:hr], alu.subtract
                    )
                    nc.vector.tensor_tensor(
                        och[:, hr:nr, :, 1], v[:, hr:nr], s4[:, hr:nr], alu.subtract
                    )
                else:
                    nc.gpsimd.tensor_tensor(v[:], s2[:], s3[:], alu.subtract)
                    nc.scalar.activation(
                        s4[:], ms[3][:], mybir.ActivationFunctionType.Identity
                    )
                    if mode == 1:
                        # near-tail: keep Pool clear for the final unit's v
                        nc.vector.tensor_tensor(
                            och[:, :, :, 1], v[:], s4[:], alu.subtract
                        )
                    else:
                        nc.gpsimd.tensor_tensor(
                            och[:, :, :, 1], v[:], s4[:], alu.subtract
                        )
                    nc.vector.tensor_tensor(och[:, :, :, 0], u[:], ms[0][:], alu.add)
                nc.sync.dma_start(
                    y_d[
                        b,
                        t * 128 : (t + 1) * 128,
                        (r0 + sr) * W : (r0 + sr + nr) * W,
                    ],
                    och[:].rearrange("p r j two -> p (r j two)"),
                )
                yield

            vt = make_v(0, 0)
            for b in range(B_LOC):
                for k in range(NCH):
                    vv = vt[:].rearrange("p v r (j two) -> p v r j two", two=2)
                    final = (b, k) == (B_LOC - 1, NCH - 1)
                    for t in range(2):
                        mode = (2 if t == 1 else 1) if final else 0
                        if mode == 2:
                            # final chunk-t as two half-units: trailing recon
                            # ops and the last store are half-size
                            hr = RCH // 2
                            for _ in chunk_t(b, k, t, vv, 0, hr, 1):
                                pass
                            for _ in chunk_t(b, k, t, vv, hr, RCH - hr, 2):
                                pass
                            continue
                        it = chunk_t(b, k, t, vv, 0, RCH, mode)
                        next(it)
                        if t == 0:
                            # prefetch next chunk's V while this chunk drains
                            nb, nk = (b, k + 1) if k + 1 < NCH else (b + 1, 0)
                            if nb < B_LOC:
                                nvt = make_v(nb, nk)
                        for u in it:
                            pass
                    vt = nvt if not final else None
    nc.finalize()
    return nc


def _bf16(a: np.ndarray) -> np.ndarray:
    return a.astype(ml_dtypes.bfloat16)


def kernel(x, weight, bias, approximate):
    """Full (unsharded) conv2d. `approximate` only selects the HW approximation
    level in the original module; the exact-math output is independent of it."""
    global _NC_CACHE, LAST_RESULTS
    x = np.ascontiguousarray(x, dtype=np.float32)
    weight = np.ascontiguousarray(weight, dtype=np.float32)
    bias = np.ascontiguousarray(bias, dtype=np.float32)

    # host: cast to bf16, zero-pad spatially; shard batch across cores
    xp = np.zeros((B, CIN, HP, WP), ml_dtypes.bfloat16)
    xp[:, :, 1 : H + 1, 1 : W + 1] = _bf16(x)
    xp = xp.reshape(B, CIN, HP * WP)

    # Winograd F(2,3) weight combos (f64 accumulate, single bf16 rounding)
    w64 = weight.astype(np.float64)  # (Cout, Cin, ky, kx)
    w0, w1, w2 = w64[:, :, :, 0], w64[:, :, :, 1], w64[:, :, :, 2]
    combos = [w0, (w0 + w1 + w2) / 2, (w0 - w1 + w2) / 2, w2]
    g = np.empty((CIN, 2, 4, 3, 128), np.float64)
    for i, cb in enumerate(combos):  # cb: (Cout, Cin, ky)
        gt = cb.transpose(1, 2, 0)  # (Cin, ky, Cout)
        g[:, 0, i, :, :] = gt[:, :, 0:128]
        g[:, 1, i, :, :] = gt[:, :, 128:256]
    g2 = np.ascontiguousarray(_bf16(g.reshape(CIN, 2 * 12 * 128)))
    b2 = bias.reshape(2, 128)

    if _NC_CACHE is None:
        _NC_CACHE = _build()
    nc = _NC_CACHE

    in_maps = [
        {"x": xp[c * B_LOC : (c + 1) * B_LOC], "g": g2, "b": b2}
        for c in range(N_CORES)
    ]
    try:
        res = run_bass_kernel_spmd(nc, in_maps, core_ids=list(range(N_CORES)))
    except Exception:
        # transient device-acquisition races (NRT_EXEC_UNIT_UNRECOVERABLE on
        # first touch after a prior process teardown) recover on retry
        import time as _time

        _time.sleep(5.0)
        res = run_bass_kernel_spmd(nc, in_maps, core_ids=list(range(N_CORES)))
    LAST_RESULTS = res
    out = np.concatenate([np.asarray(r["y"]) for r in res.results], axis=0)
    return out.astype(np.float32).reshape(B, COUT, H, W)


# revision 42
# speedup vs baseline: 1.0063x; 1.0008x over previous
"""Trainium2 Bass kernel for nn_CustomConv2d: 3x3 conv, stride 1, pad 1.

Full shapes: x (32,128,56,56) f32, weight (256,128,3,3) f32, bias (256,) f32.
Output: (32,256,56,56) f32.

Strategy: data-parallel over batch (8 cores x 4 images) + 1D Winograd F(2,3)
along W in bf16. Per output column-pair only 4 products are needed instead of
6, cutting PE matmul work to 2/3 of the direct conv: per (image, cout-half,
14-row chunk) the kernel runs 12 accumulating bf16 matmuls (4 Winograd
components x 3 ky taps, contraction = Cin = 128, free = 14 rows x 28 pairs =
392) instead of 9 direct taps over all 56 columns. Input transform (E/T/U
full-width row combos) runs on DVE at the 2x bf16-packed rate. Reconstruction
(out0 = m1+m2+m3+bias at even cols, out1 = m2-m3-m4+bias at odd) respects the
engine PSUM rules (DVE max one PSUM input, Pool none): ACT drains s2=m2+bias,
s3=m3, s4=m4 to SBUF bf16, DVE computes u=s2+s3 and the single trailing op
och0=u+m1, Pool computes v=s2-s3 and och1=v-s4. Group order m2,m3,m4,m1 lets
recon overlap the matmul stream. x is cast to bf16 on the host (halves input
DMA); outputs store as bf16 and upcast on the host (halves output DMA); G
combos live in a [cin][t][12][128] layout so every weight DMA is contiguous
(strided DMAs cost ~2x per byte). Max rel err 7.0e-3 vs the f32 reference
(gate 2e-2). Dep-free warmup matmuls bridge the initial DMA wait and the PE
clock ramp; the final chunk-t runs as two half-units so the trailing recon
and last store are half-size. Cost-model timeline: 71906 ns (baseline
direct-conv f32r: 104438).
"""

import numpy as np
import ml_dtypes

import concourse.bass as bass
import concourse.mybir as mybir
import concourse.tile as tile
from concourse import bacc
from concourse.bass_utils import run_bass_kernel_spmd

N_CORES = 8
B = 32
B_LOC = B // N_CORES  # 4
CIN = 128
COUT = 256
H = W = 56
HP = WP = 58  # padded
RCH = 14  # output rows per chunk
NCH = H // RCH  # 4
J = W // 2  # 28 column pairs
NWARM = 8

_NC_CACHE = None
LAST_RESULTS = None  # stashed BassKernelResults for test harness introspection


def _build(reps: int = 1) -> bass.Bass:
    f32 = mybir.dt.float32
    bf16 = mybir.dt.bfloat16
    alu = mybir.AluOpType
    nc = bacc.Bacc(None, target_bir_lowering=False)
    x_d = nc.dram_tensor("x", [B_LOC, CIN, HP * WP], bf16, kind="ExternalInput")
    g_d = nc.dram_tensor("g", [CIN, 2 * 12 * 128], bf16, kind="ExternalInput")
    b_d = nc.dram_tensor("b", [2, 128], f32, kind="ExternalInput")
    y_d = nc.dram_tensor("y", [B_LOC, COUT, H * W], bf16, kind="ExternalOutput")

    g3 = g_d[:].rearrange("p (t i o) -> p t i o", t=2, i=12)

    from contextlib import ExitStack, nullcontext

    with tile.TileContext(nc) as tc, ExitStack() as es:
        cpool = es.enter_context(tc.tile_pool(name="const", bufs=1))
        xpool = es.enter_context(tc.tile_pool(name="xp", bufs=B_LOC))
        vpool = es.enter_context(tc.tile_pool(name="vp", bufs=3))
        tpool = es.enter_context(tc.tile_pool(name="tmp", bufs=4))
        opool = es.enter_context(tc.tile_pool(name="out", bufs=6))
        pspool = es.enter_context(tc.tile_pool(name="ps", bufs=8, space="PSUM"))
        with tc.For_i(0, reps, 1) if reps > 1 else nullcontext():
            wtile = cpool.tile([CIN, 2, 12, 128], bf16)
            btile = cpool.tile([128, 2], f32)
            xpads = [
                xpool.tile([CIN, HP, WP], bf16, tag="xpad", name=f"xpad{i}")
                for i in range(B_LOC)
            ]

            # PE warmup: dep-free matmuls bridge the initial DMA wait and
            # bring the PE clock (HAM) to full rate before the real work.
            wsrc = cpool.tile([128, RCH * J], bf16)
            nc.gpsimd.memset(wsrc[:], 0.0)
            wps = pspool.tile([128, RCH * J], f32, tag="m")
            for _ in range(NWARM):
                nc.tensor.matmul(
                    wps[0:64, :], wsrc[:, 0:64], wsrc[:], start=True, stop=True
                )

            # DMA issue order = criticality: chunk0's V transform needs xpad
            # rows 0..15 first; the first matmul group needs G[:, 0:3, 0:128].
            xsrc0 = x_d[0].rearrange("p (h w) -> p h w", h=HP)
            nc.sync.dma_start(xpads[0][:, 0:16, :], xsrc0[:, 0:16, :])
            nc.sync.dma_start(wtile[:, 0, 3:12, :], g3[:, 0, 3:12, :])
            nc.sync.dma_start(wtile[:, 0, 0:3, :], g3[:, 0, 0:3, :])
            nc.sync.dma_start(wtile[:, 1], g3[:, 1])
            nc.sync.dma_start(btile[:], b_d[:].rearrange("t p -> p t"))
            nc.sync.dma_start(xpads[0][:, 16:37, :], xsrc0[:, 16:37, :])
            nc.sync.dma_start(xpads[0][:, 37:58, :], xsrc0[:, 37:58, :])
            for b in range(1, B_LOC):
                xsrc = x_d[b].rearrange("p (h w) -> p h w", h=HP)
                nc.sync.dma_start(xpads[b][:, 0:29, :], xsrc[:, 0:29, :])
                nc.sync.dma_start(xpads[b][:, 29:58, :], xsrc[:, 29:58, :])

            def make_v(b, k):
                """E/T/U row transforms for chunk k of image b (DVE, bf16 2x)."""
                xp = xpads[b]
                r0 = k * RCH
                vt = vpool.tile([CIN, 3, RCH + 2, W], bf16, tag="v")
                rs = slice(r0, r0 + RCH + 2)
                # issue order matches first use: T (m2 group runs first),
                # then U (m3), then E (m4/m1)
                nc.vector.tensor_tensor(
                    vt[:, 1], xp[:, rs, 1:57], xp[:, rs, 2:58], alu.add
                )
                nc.vector.tensor_tensor(
                    vt[:, 2], xp[:, rs, 2:58], xp[:, rs, 1:57], alu.subtract
                )
                nc.vector.tensor_tensor(
                    vt[:, 0], xp[:, rs, 0:56], xp[:, rs, 2:58], alu.subtract
                )
                return vt

            # (m-index, v-component, column phase); group order m2,m3,m4,m1:
            # ACT drains s2 = m2+bias and s3 = m3 to SBUF mid-stream (the HW
            # allows only ONE PSUM input per vector op), DVE/Pool combine
            # u = s2+s3, v = s2-s3, och1 = v-m4; only och0 = u+m1 trails.
            SRCS = [(1, 1, 0), (2, 2, 0), (3, 0, 1), (0, 0, 0)]

            def chunk_t(b, k, t, vv, sr, nr, mode):
                """One matmul+recon unit over rows sr..sr+nr of chunk k.
                mode 2 = final unit (recon tuned for trailing latency)."""
                r0 = k * RCH
                ms = [None] * 4
                for i, vc, ph in SRCS:
                    ms[i] = pspool.tile(
                        [CIN, nr, J], f32, tag="m", name=f"m{b}_{k}_{t}_{sr}_{i}"
                    )
                    for ky in range(3):
                        nc.tensor.matmul(
                            ms[i][:],
                            wtile[:, t, i * 3 + ky, :],
                            vv[:, vc, sr + ky : sr + ky + nr, :, ph],
                            start=(ky == 0),
                            stop=(ky == 2),
                        )
                yield  # caller interleaves V prefetch here
                och = opool.tile([128, nr, J, 2], bf16, tag="och")
                s2 = tpool.tile([128, nr, J], bf16, tag="s2")
                s3 = tpool.tile([128, nr, J], bf16, tag="s3")
                u = tpool.tile([128, nr, J], bf16, tag="u")
                v = tpool.tile([128, nr, J], bf16, tag="vv")
                bias = btile[:, t : t + 1]
                nc.scalar.activation(
                    s2[:], ms[1][:], mybir.ActivationFunctionType.Identity, bias=bias
                )
                nc.scalar.activation(
                    s3[:], ms[2][:], mybir.ActivationFunctionType.Identity
                )
                nc.vector.tensor_tensor(u[:], s2[:], s3[:], alu.add)
                s4 = tpool.tile([128, nr, J], bf16, tag="s4")
                if mode == 2:
                    # final unit: v early on Pool (mode 1 freed it), och0 on
                    # DVE, och1 split Pool || DVE after the last matmul
                    hr = nr // 2
                    nc.gpsimd.tensor_tensor(v[:], s2[:], s3[:], alu.subtract)
                    nc.scalar.activation(
                        s4[:], ms[3][:], mybir.ActivationFunctionType.Identity
                    )
                    nc.vector.tensor_tensor(och[:, :, :, 0], u[:], ms[0][:], alu.add)
                    nc.gpsimd.tensor_tensor(
                        och[:, 0:hr, :, 1], v[:, 0:hr], s4[:, 0:hr], alu.subtract
                    )
                    nc.vector.tensor_tensor(
                        och[:, hr:nr, :, 1], v[:, hr:nr], s4[:, hr:nr], alu.subtract
                    )
                else:
                    nc.gpsimd.tensor_tensor(v[:], s2[:], s3[:], alu.subtract)
                    nc.scalar.activation(
                        s4[:], ms[3][:], mybir.ActivationFunctionType.Identity
                    )
                    if mode == 1:
                        # near-tail: keep Pool clear for the final unit's v
                        nc.vector.tensor_tensor(
                            och[:, :, :, 1], v[:], s4[:], alu.subtract
                        )
                    else:
                        nc.gpsimd.tensor_tensor(
                            och[:, :, :, 1], v[:], s4[:], alu.subtract
                        )
                    nc.vector.tensor_tensor(och[:, :, :, 0], u[:], ms[0][:], alu.add)
                nc.sync.dma_start(
                    y_d[
                        b,
                        t * 128 : (t + 1) * 128,
                        (r0 + sr) * W : (r0 + sr + nr) * W,
                    ],
                    och[:].rearrange("p r j two -> p (r j two)"),
                )
                yield

            vt = make_v(0, 0)
            for b in range(B_LOC):
                for k in range(NCH):
                    vv = vt[:].rearrange("p v r (j two) -> p v r j two", two=2)
                    final = (b, k) == (B_LOC - 1, NCH - 1)
                    for t in range(2):
                        mode = (2 if t == 1 else 1) if final else 0
                        if mode == 2:
                            # final chunk-t as two half-units: trailing recon
                            # ops and the last store are half-size
                            hr = 8
                            for _ in chunk_t(b, k, t, vv, 0, hr, 1):
                                pass
                            for _ in chunk_t(b, k, t, vv, hr, RCH - hr, 2):
                                pass
                            continue
                        it = chunk_t(b, k, t, vv, 0, RCH, mode)
                        next(it)
                        if t == 0:
                            # prefetch next chunk's V while this chunk drains
                            nb, nk = (b, k + 1) if k + 1 < NCH else (b + 1, 0)
                            if nb < B_LOC:
                                nvt = make_v(nb, nk)
                        for u in it:
                            pass
                    vt = nvt if not final else None
    nc.finalize()
    return nc


def _bf16(a: np.ndarray) -> np.ndarray:
    return a.astype(ml_dtypes.bfloat16)


def kernel(x, weight, bias, approximate):
    """Full (unsharded) conv2d. `approximate` only selects the HW approximation
    level in the original module; the exact-math output is independent of it."""
    global _NC_CACHE, LAST_RESULTS
    x = np.ascontiguousarray(x, dtype=np.float32)
    weight = np.ascontiguousarray(weight, dtype=np.float32)
    bias = np.ascontiguousarray(bias, dtype=np.float32)

    # host: cast to bf16, zero-pad spatially; shard batch across cores
    xp = np.zeros((B, CIN, HP, WP), ml_dtypes.bfloat16)
    xp[:, :, 1 : H + 1, 1 : W + 1] = _bf16(x)
    xp = xp.reshape(B, CIN, HP * WP)

    # Winograd F(2,3) weight combos (f64 accumulate, single bf16 rounding)
    w64 = weight.astype(np.float64)  # (Cout, Cin, ky, kx)
    w0, w1, w2 = w64[:, :, :, 0], w64[:, :, :, 1], w64[:, :, :, 2]
    combos = [w0, (w0 + w1 + w2) / 2, (w0 - w1 + w2) / 2, w2]
    g = np.empty((CIN, 2, 4, 3, 128), np.float64)
    for i, cb in enumerate(combos):  # cb: (Cout, Cin, ky)
        gt = cb.transpose(1, 2, 0)  # (Cin, ky, Cout)
        g[:, 0, i, :, :] = gt[:, :, 0:128]
        g[:, 1, i, :, :] = gt[:, :, 128:256]
    g2 = np.ascontiguousarray(_bf16(g.reshape(CIN, 2 * 12 * 128)))
    b2 = bias.reshape(2, 128)

    if _NC_CACHE is None:
        _NC_CACHE = _build()
    nc = _NC_CACHE

    in_maps = [
        {"x": xp[c * B_LOC : (c + 1) * B_LOC], "g": g2, "b": b2}
        for c in range(N_CORES)
    ]
    try:
        res = run_bass_kernel_spmd(nc, in_maps, core_ids=list(range(N_CORES)))
    except Exception:
        # transient device-acquisition races (NRT_EXEC_UNIT_UNRECOVERABLE on
        # first touch after a prior process teardown) recover on retry
        import time as _time

        _time.sleep(5.0)
        res = run_bass_kernel_spmd(nc, in_maps, core_ids=list(range(N_CORES)))
    LAST_RESULTS = res
    out = np.concatenate([np.asarray(r["y"]) for r in res.results], axis=0)
    return out.astype(np.float32).reshape(B, COUT, H, W)
